# revision 1
# baseline (speedup 1.0000x reference)
"""Trainium2 Bass kernel for nn_CBDAE (2-layer GRU encoder + contrastive head
+ autoregressive 2-layer GRU decoder).

Strategy (see spec sharding hint): data-parallel over batch B=128 across 8
cores (16 rows each), all weights replicated.  Per core, every matmul runs
weights-stationary (lhsT = W^T tile [K=128, M=128], bf16 so FastWeightLoad
engages) with the per-core batch (16) as the moving free dim.  Gates are
produced TRANSPOSED ([gate-dim partitions, batch free]) so the elementwise
GRU math runs across 128 partitions.  Input (Wih) and hidden (Whh) matmuls
go to separate PSUM column regions so the PE never stalls mid-accumulation
on the recurrence's serial chain; the decoder output projection is software
-rotated by one step so its PSUM->y chain hides under the next step's
hidden matmuls.
"""

import numpy as np
import ml_dtypes

import concourse.bass as bass
import concourse.mybir as mybir
from concourse import bacc
from concourse.bass import ds
from concourse.bass_utils import run_bass_kernel_spmd
from concourse.tile import TileContext

# ---- problem shapes (hardcoded per contest contract) ----
B, T, N = 128, 512, 128
H, G1, G = 512, 256, 128
NCORES = 8
BL = B // NCORES          # 16 local batch rows per core
P = 128                   # partitions
H3 = 3 * H                # 1536
NM = H3 // P              # 12 m-tiles per gate matmul
NKH = H // P              # 4 k-tiles for H-dim contraction
U = 4                     # loop unroll (steps per For_i body)

F32 = mybir.dt.float32
BF16 = mybir.dt.bfloat16

Sigmoid = mybir.ActivationFunctionType.Sigmoid
Tanh = mybir.ActivationFunctionType.Tanh
Relu = mybir.ActivationFunctionType.Relu
Identity = mybir.ActivationFunctionType.Identity

# ---------------------------------------------------------------------------
# weight blob layout: sequence of [128,128] lhsT tiles, index -> col offset
# ---------------------------------------------------------------------------
_SEGS = [
    ("enc0_ih", 1, NM), ("enc0_hh", NKH, NM),
    ("enc1_ih", NKH, NM), ("enc1_hh", NKH, NM),
    ("dec0_ih", 1, NM), ("dec0_hh", NKH, NM),
    ("dec1_ih", NKH, NM), ("dec1_hh", NKH, NM),
    ("wout", NKH, 1), ("wg1", NKH, 2), ("wg2", 2, 1),
]


def _seg_bases():
    bases, idx = {}, 0
    for name, nk, nm in _SEGS:
        bases[name] = idx
        idx += nk * nm
    return bases, idx


_BASES, _NTILES = _seg_bases()
WCOLS = _NTILES * P
BCOLS = 4 * 256 + 1   # 4 gru layers x (rz_rep 128 | bihn 64 | bhhn 64) + wout_b


def _wofs(seg, k, m):
    name, nk, nm = next(s for s in _SEGS if s[0] == seg)
    assert k < nk and m < nm
    return (_BASES[seg] + k * nm + m) * P


# ---------------------------------------------------------------------------
# host-side packing
# ---------------------------------------------------------------------------
def _pack_weight(blob, seg, W):
    """W: [out_dim, in_dim] fp32 -> lhsT tiles into blob."""
    lhsT = np.ascontiguousarray(W.T)          # [in, out]
    name, nk, nm = next(s for s in _SEGS if s[0] == seg)
    assert lhsT.shape == (nk * P, nm * P), (seg, lhsT.shape)
    for k in range(nk):
        for m in range(nm):
            c = _wofs(seg, k, m)
            blob[:, c:c + P] = lhsT[k * P:(k + 1) * P, m * P:(m + 1) * P]


def _rep_vec(vec):
    """[n_cols_groups*128] -> [128, n*16] with col 16j+b = vec[128j+p]."""
    ngr = vec.shape[0] // P
    t = vec.reshape(ngr, P).T                 # [128, ngr]
    return np.repeat(t[:, :, None], BL, axis=2).reshape(P, ngr * BL)


def _pack_bias(bias_blob, li, bih, bhh):
    base = li * 256
    bias_blob[:, base:base + 128] = _rep_vec(bih[:2 * H] + bhh[:2 * H])
    bias_blob[:, base + 128:base + 192] = _rep_vec(bih[2 * H:])
    bias_blob[:, base + 192:base + 256] = _rep_vec(bhh[2 * H:])


def _host_blobs(inputs):
    wblob = np.zeros((P, WCOLS), np.float32)
    _pack_weight(wblob, "enc0_ih", inputs["enc_Wih0"])
    _pack_weight(wblob, "enc0_hh", inputs["enc_Whh0"])
    _pack_weight(wblob, "enc1_ih", inputs["enc_Wih1"])
    _pack_weight(wblob, "enc1_hh", inputs["enc_Whh1"])
    _pack_weight(wblob, "dec0_ih", inputs["dec_Wih0"])
    _pack_weight(wblob, "dec0_hh", inputs["dec_Whh0"])
    _pack_weight(wblob, "dec1_ih", inputs["dec_Wih1"])
    _pack_weight(wblob, "dec1_hh", inputs["dec_Whh1"])
    _pack_weight(wblob, "wout", inputs["Wout_w"])
    _pack_weight(wblob, "wg1", inputs["Wg1"])
    _pack_weight(wblob, "wg2", inputs["Wg2"])
    wblob = wblob.astype(ml_dtypes.bfloat16)

    bblob = np.zeros((P, BCOLS), np.float32)
    _pack_bias(bblob, 0, inputs["enc_bih0"], inputs["enc_bhh0"])
    _pack_bias(bblob, 1, inputs["enc_bih1"], inputs["enc_bhh1"])
    _pack_bias(bblob, 2, inputs["dec_bih0"], inputs["dec_bhh0"])
    _pack_bias(bblob, 3, inputs["dec_bih1"], inputs["dec_bhh1"])
    bblob[:, 1024] = inputs["Wout_b"]

    # decoder step-0 trick: out-proj of "step -1" must produce y == 0.
    # Find d with Wout_w @ d + Wout_b ~= 0 in the bf16 lattice.
    Wo = inputs["Wout_w"].astype(np.float64)
    bo = inputs["Wout_b"].astype(np.float64)
    d = -np.linalg.lstsq(Wo, bo, rcond=None)[0]
    for _ in range(3):
        dbf = d.astype(ml_dtypes.bfloat16).astype(np.float64)
        r = Wo @ dbf + bo
        d = dbf - np.linalg.lstsq(Wo, r, rcond=None)[0]
    dbf = d.astype(np.float32).astype(ml_dtypes.bfloat16)
    d1o = np.repeat(dbf.reshape(NKH, P).T[:, :, None], BL, axis=2).reshape(P, NKH * BL)

    return wblob, bblob, d1o


# ---------------------------------------------------------------------------
# device program
# ---------------------------------------------------------------------------
_CACHE = {}


def _build():
    if "nc" in _CACHE:
        return _CACHE["nc"]

    nc = bacc.Bacc("TRN2", target_bir_lowering=False, debug=False,
                   enable_asserts=False, num_devices=NCORES)

    xT = nc.dram_tensor("xT", [P, BL * T], BF16, kind="ExternalInput").ap()
    Wd = nc.dram_tensor("Wd", [P, WCOLS], BF16, kind="ExternalInput").ap()
    Bd = nc.dram_tensor("Bd", [P, BCOLS], F32, kind="ExternalInput").ap()
    D1od = nc.dram_tensor("D1od", [P, NKH * BL], BF16, kind="ExternalInput").ap()
    y_out = nc.dram_tensor("y_out", [P, BL * (T + 1)], F32, kind="ExternalOutput").ap()
    z_out = nc.dram_tensor("z_out", [P, BL], F32, kind="ExternalOutput").ap()

    with TileContext(nc) as tc:
        with (
            tc.tile_pool(name="singles", bufs=1) as singles,
            tc.tile_pool(name="work", bufs=3) as work,
            tc.tile_pool(name="psum", bufs=2, space="PSUM") as psum_pool,
        ):
            Wsb = singles.tile([P, WCOLS], BF16)
            xs = singles.tile([P, BL * T], BF16)
            Bsb = singles.tile([P, BCOLS], F32)
            h1f = singles.tile([P, NKH * BL], F32)
            h2f = singles.tile([P, NKH * BL], F32)
            h1b = singles.tile([P, NKH * BL], BF16)
            h2b = singles.tile([P, NKH * BL], BF16)
            yb = singles.tile([P, BL], BF16)
            d1o = singles.tile([P, NKH * BL], BF16)

            nc.sync.dma_start(out=Wsb, in_=Wd)
            nc.sync.dma_start(out=xs, in_=xT)
            nc.sync.dma_start(out=Bsb, in_=Bd)
            nc.sync.dma_start(out=d1o, in_=D1od)
            nc.vector.memset(h1f, 0.0)
            nc.vector.memset(h2f, 0.0)
            nc.vector.memset(h1b, 0.0)
            nc.vector.memset(h2b, 0.0)

            def wt(seg, k, m):
                c = _wofs(seg, k, m)
                return Wsb[:, c:c + P]

            # psum regions within one [128,384] bank tile:
            #   gh (Whh): rz 0:128, n 128:192 ; gi (Wih): rz 192:320, n 320:384
            def gru_hh(ps, seg, hb_src):
                for m in range(NM):
                    col = 16 * m if m < 8 else 128 + 16 * (m - 8)
                    for k in range(NKH):
                        nc.tensor.matmul(
                            ps[:, col:col + BL], wt(seg, k, m),
                            hb_src[:, k * BL:(k + 1) * BL],
                            start=(k == 0), stop=(k == NKH - 1))

            def gru_ih(ps, seg, nk, rhs_fn):
                for m in range(NM):
                    col = 192 + 16 * m if m < 8 else 320 + 16 * (m - 8)
                    for k in range(nk):
                        nc.tensor.matmul(
                            ps[:, col:col + BL], wt(seg, k, m), rhs_fn(k),
                            start=(k == 0), stop=(k == nk - 1))

            def gru_chain(ps, li, hf, hb, extra=None):
                bb = li * 256
                t1 = work.tile([P, 128], F32, tag="t1")
                nc.vector.tensor_add(t1, ps[:, 0:128], Bsb[:, bb:bb + 128])
                t2 = work.tile([P, 128], F32, tag="t2")
                nc.vector.tensor_add(t2, t1, ps[:, 192:320])
                rz = work.tile([P, 128], F32, tag="rz")
                nc.scalar.activation(rz, t2, Sigmoid)
                hn = work.tile([P, 64], F32, tag="hn")
                nc.vector.tensor_add(hn, ps[:, 128:192], Bsb[:, bb + 192:bb + 256])
                u = work.tile([P, 64], F32, tag="u")
                nc.vector.tensor_mul(u, rz[:, 0:64], hn)
                v = work.tile([P, 64], F32, tag="v")
                nc.vector.tensor_add(v, u, ps[:, 320:384])
                w = work.tile([P, 64], F32, tag="w")
                nc.vector.tensor_add(w, v, Bsb[:, bb + 128:bb + 192])
                nn = work.tile([P, 64], F32, tag="nn")
                nc.scalar.activation(nn, w, Tanh)
                hmn = work.tile([P, 64], F32, tag="hmn")
                nc.vector.tensor_sub(hmn, hf, nn)
                zm = work.tile([P, 64], F32, tag="zm")
                nc.vector.tensor_mul(zm, rz[:, 64:128], hmn)
                nc.vector.tensor_add(hf, nn, zm)
                nc.vector.tensor_copy(hb, hf)
                if extra is not None:
                    nc.vector.tensor_copy(extra, hf)

            # ---------------- encoder ----------------
            with tc.For_i(0, BL * T, BL * U, hint_engines=(mybir.EngineType.PE,)) as iv:
                for u in range(U):
                    col = iv + BL * u
                    ps0 = psum_pool.tile([P, 384], F32, tag="ps0")
                    gru_hh(ps0, "enc0_hh", h1b)
                    gru_ih(ps0, "enc0_ih", 1, lambda k: xs[:, ds(col, BL)])
                    ps1 = psum_pool.tile([P, 384], F32, tag="ps1")
                    gru_hh(ps1, "enc1_hh", h2b)
                    gru_chain(ps0, 0, h1f, h1b)
                    gru_ih(ps1, "enc1_ih", NKH,
                           lambda k: h1b[:, k * BL:(k + 1) * BL])
                    gru_chain(ps1, 1, h2f, h2b)

            # ---------------- contrastive head ----------------
            hps = psum_pool.tile([P, 384], F32, tag="ps0")
            for m in range(2):
                for k in range(NKH):
                    nc.tensor.matmul(hps[:, 16 * m:16 * m + BL], wt("wg1", k, m),
                                     h2b[:, k * BL:(k + 1) * BL],
                                     start=(k == 0), stop=(k == NKH - 1))
            ab = work.tile([P, 2 * BL], BF16, tag="ab")
            nc.scalar.activation(ab, hps[:, 0:2 * BL], Relu)
            for k in range(2):
                nc.tensor.matmul(hps[:, 48:48 + BL], wt("wg2", k, 0),
                                 ab[:, k * BL:(k + 1) * BL],
                                 start=(k == 0), stop=(k == 1))
            zsb = work.tile([P, BL], F32, tag="zsb")
            nc.vector.tensor_copy(zsb, hps[:, 48:48 + BL])
            nc.sync.dma_start(out=z_out, in_=zsb)

            # ---------------- decoder (out-proj rotated by one step) --------
            with tc.For_i(0, BL * T, BL * U, hint_engines=(mybir.EngineType.PE,)) as iv:
                for u in range(U):
                    col = iv + BL * u
                    ps0 = psum_pool.tile([P, 384], F32, tag="ps0")
                    gru_hh(ps0, "dec0_hh", h1b)
                    # out-projection of previous step (reads d1o)
                    psy = psum_pool.tile([P, BL], F32, tag="psy")
                    for k in range(NKH):
                        nc.tensor.matmul(psy, wt("wout", k, 0),
                                         d1o[:, k * BL:(k + 1) * BL],
                                         start=(k == 0), stop=(k == NKH - 1))
                    ysb = work.tile([P, BL], F32, tag="ysb")
                    nc.scalar.activation(ysb, psy, Identity,
                                         bias=Bsb[:, 1024:1025])
                    nc.vector.tensor_copy(yb, ysb)
                    nc.sync.dma_start(out=y_out[:, ds(col, BL)], in_=ysb)
                    gru_ih(ps0, "dec0_ih", 1, lambda k: yb[:, 0:BL])
                    gru_chain(ps0, 2, h1f, h1b)
                    ps1 = psum_pool.tile([P, 384], F32, tag="ps1")
                    gru_hh(ps1, "dec1_hh", h2b)
                    gru_ih(ps1, "dec1_ih", NKH,
                           lambda k: h1b[:, k * BL:(k + 1) * BL])
                    gru_chain(ps1, 3, h2f, h2b, extra=d1o)

            # final output projection (step T-1)
            psy = psum_pool.tile([P, BL], F32, tag="psy")
            for k in range(NKH):
                nc.tensor.matmul(psy, wt("wout", k, 0),
                                 d1o[:, k * BL:(k + 1) * BL],
                                 start=(k == 0), stop=(k == NKH - 1))
            ysb = work.tile([P, BL], F32, tag="ysb")
            nc.scalar.activation(ysb, psy, Identity, bias=Bsb[:, 1024:1025])
            nc.sync.dma_start(out=y_out[:, BL * T:BL * (T + 1)], in_=ysb)

    nc.finalize()
    _CACHE["nc"] = nc
    return nc


# ---------------------------------------------------------------------------
# entry point
# ---------------------------------------------------------------------------
def kernel(**inputs):
    inputs = {k: np.asarray(v, np.float32 if np.asarray(v).dtype != np.int32
                            else np.int32) for k, v in inputs.items()}
    nc = _build()
    wblob, bblob, d1o = _host_blobs(inputs)

    x = inputs["x"]                                  # [B, T, N]
    in_maps = []
    for c in range(NCORES):
        xs = x[c * BL:(c + 1) * BL]                  # [16, T, N]
        xT = np.ascontiguousarray(xs.transpose(2, 1, 0).reshape(P, T * BL))
        in_maps.append({
            "xT": xT.astype(ml_dtypes.bfloat16),
            "Wd": wblob, "Bd": bblob, "D1od": d1o,
        })

    res = run_bass_kernel_spmd(nc, in_maps, core_ids=list(range(NCORES)))

    zT = np.zeros((B, G), np.float32)
    y_hat = np.zeros((B, T, N), np.float32)
    for c in range(NCORES):
        om = res.results[c]
        zT[c * BL:(c + 1) * BL] = om["z_out"].T
        yT = om["y_out"][:, BL:]                     # drop rotated col 0
        y_hat[c * BL:(c + 1) * BL] = yT.reshape(P, T, BL).transpose(2, 1, 0)
    return zT, y_hat


if __name__ == "__main__":
    rng = np.random.default_rng(0)
    fake = {"x": rng.standard_normal((B, T, N), np.float32)}
    for pre in ("enc", "dec"):
        for l, din in enumerate((N, H)):
            fake[f"{pre}_Wih{l}"] = rng.standard_normal((H3, din), np.float32) * 0.02
            fake[f"{pre}_Whh{l}"] = rng.standard_normal((H3, H), np.float32) * 0.02
            fake[f"{pre}_bih{l}"] = rng.standard_normal(H3).astype(np.float32) * 0.02
            fake[f"{pre}_bhh{l}"] = rng.standard_normal(H3).astype(np.float32) * 0.02
    fake["Wg1"] = rng.standard_normal((G1, H), np.float32) * 0.02
    fake["Wg2"] = rng.standard_normal((G, G1), np.float32) * 0.02
    fake["Wout_w"] = rng.standard_normal((N, H), np.float32) * 0.02
    fake["Wout_b"] = rng.standard_normal(N).astype(np.float32) * 0.02
    z, y = kernel(**fake)
    print("kernel ran:", z.shape, y.shape)
